# revision 16
# baseline (speedup 1.0000x reference)
"""Trainium2 Bass kernel for nn_BasicBlock_77137612636387.

TCN encoder (last-timestep only) + two autoregressive GRU decoders.
Pure data parallel over 8 NeuronCores: batch 1024 -> 128 per core, all
weights replicated. bf16 matmuls with fp32 PSUM accumulation; gate math
fp32 on ACT/DVE.

Layout choices (per core, BS=128):
 - GRU state h kept bf16 in [batch_part, H] ("row" layout) and, via PE
   transposes, as hT [H_part(8 blocks of 128), batch] which serves as the
   matmul stationary operand (lhsT).
 - Gate pre-activations ga = [h,x,1] @ W_aug computed per 128-wide H-block
   into one PSUM bank: 512 cols = [r_b | z_b | hn_b | in_b] (W columns are
   host-permuted). x (scalar GRU input) and all biases enter through a K=2
   matmul with lhsT = [x_row; ones_row].
 - Per block: r=sig(ps0), z=sig(ps1), n=tanh(r*ps2+ps3), h'=n+z*(h-n),
   PE-transpose of h' block -> next step's lhsT.
 - pred (scalar output, next x) = fc @ hT via 8 [128,1]-lhsT matmuls.
"""
import os
from contextlib import ExitStack

import numpy as np
import ml_dtypes

import bass_rust
import concourse.bass as bass
import concourse.tile as tile
from concourse import mybir
from concourse.bass_utils import run_bass_kernel_spmd

AF = mybir.ActivationFunctionType
F32 = mybir.dt.float32
BF16 = mybir.dt.bfloat16

B, L, F, H, KW = 1024, 168, 8, 1024, 3
NCORES = 8
BS = B // NCORES          # 128 batch per core
NB = H // 128             # 8 H-blocks
T_FORE = int(os.environ.get("KB_T_FORE", "24"))
T_EST = int(os.environ.get("KB_T_EST", "168"))
INTERLEAVE = os.environ.get("KB_INTERLEAVE", "1") == "1"

bf = ml_dtypes.bfloat16


def _perm3():
    """Column permutation: block b -> [r_b, z_b, n_b] (each 128 wide)."""
    idx = []
    for b in range(NB):
        idx.extend(range(b * 128, (b + 1) * 128))            # r
        idx.extend(range(H + b * 128, H + (b + 1) * 128))    # z
        idx.extend(range(2 * H + b * 128, 2 * H + (b + 1) * 128))  # n
    return np.array(idx)


def _prep_gru(w_ih, w_hh, b_ih, b_hh, fc_w):
    perm = _perm3()
    # Wh: [8(k), 128, 3072] — rhs slices [:,k,b*384:(b+1)*384] vs psum[:,0:384]
    Wh = w_hh.T[:, perm].reshape(NB, 128, 3 * H).astype(bf)
    # Wxb: [2, 4096] — per block b, cols b*512+0:384 = rzn (x-weights w/ n
    # zeroed + combined biases), cols b*512+384:512 = i_n (w_ih_n, b_ih_n)
    q = w_ih[:, 0].copy()
    q[2 * H:] = 0.0
    bb = (b_ih + b_hh).copy()
    bb[2 * H:] = b_hh[2 * H:]
    Wxb = np.zeros((2, 4 * H), np.float32)
    for b in range(NB):
        Wxb[0, b * 512:b * 512 + 384] = q[perm[b * 384:(b + 1) * 384]]
        Wxb[1, b * 512:b * 512 + 384] = bb[perm[b * 384:(b + 1) * 384]]
        Wxb[0, b * 512 + 384:(b + 1) * 512] = w_ih[2 * H + b * 128:2 * H + (b + 1) * 128, 0]
        Wxb[1, b * 512 + 384:(b + 1) * 512] = b_ih[2 * H + b * 128:2 * H + (b + 1) * 128]
    fc = fc_w[0].reshape(NB, 128).astype(bf)  # [8(k), 128]
    return Wh, Wxb.astype(bf), fc


def _prep_weights(p):
    w = {}
    w["wh_f"], w["wxb_f"], w["fc_f"] = _prep_gru(p["f_w_ih"], p["f_w_hh"], p["f_b_ih"], p["f_b_hh"], p["f_fc_w"])
    w["wh_e"], w["wxb_e"], w["fc_e"] = _prep_gru(p["e_w_ih"], p["e_w_hh"], p["e_b_ih"], p["e_b_hh"], p["e_fc_w"])
    # conv1 lhsT: [24, 1024], row j = k*8+f (time-major to match the DMA'd
    # window tiles), col = out channel
    w1t = np.zeros((24, H), np.float32)
    for k in range(KW):
        for f in range(F):
            w1t[k * F + f, :] = p["conv1_w"][:, f, k]
    w["w1t"] = w1t.astype(bf)
    # conv2 rhs: [3, 8(kb), 128, 1024]: [tap, kb, p, n] = conv2_w[n, kb*128+p, tap]
    w2 = np.transpose(p["conv2_w"], (2, 1, 0))  # [tap, in, out]
    w["w2r"] = w2.reshape(KW, NB, 128, H).astype(bf)
    w["downt"] = p["down_w"][:, :, 0].T.astype(bf)      # [8, 1024]
    w["b1"] = p["conv1_b"].reshape(NB, 128).astype(np.float32)  # [8(m), 128]
    w["b2row"] = p["conv2_b"][None, :].astype(bf)       # [1, 1024]
    w["bdrow"] = p["down_b"][None, :].astype(bf)        # [1, 1024]
    w["ident"] = np.eye(128, dtype=bf)
    return w


def _build(fcb_f: float, fcb_e: float):
    nc = bass.Bass()

    x_in = nc.dram_tensor("x_in", [BS, L, F], F32, kind="ExternalInput")
    dr = {}
    for name, shape, dt in [
        ("wh_f", [NB, 128, 3 * H], BF16), ("wxb_f", [2, 4 * H], BF16), ("fc_f", [NB, 128], BF16),
        ("wh_e", [NB, 128, 3 * H], BF16), ("wxb_e", [2, 4 * H], BF16), ("fc_e", [NB, 128], BF16),
        ("w1t", [24, H], BF16), ("w2r", [KW, NB, 128, H], BF16), ("downt", [F, H], BF16),
        ("b1", [NB, 128], F32), ("b2row", [1, H], BF16), ("bdrow", [1, H], BF16),
        ("ident", [128, 128], BF16),
    ]:
        dr[name] = nc.dram_tensor(name, shape, dt, kind="ExternalInput")
    fore_o = nc.dram_tensor("fore", [T_FORE, BS], F32, kind="ExternalOutput")
    est_o = nc.dram_tensor("esto", [T_EST, BS], F32, kind="ExternalOutput")
    predse_raw = nc.dram_tensor("predse_raw", [T_EST, BS], F32)

    with tile.TileContext(nc) as tc, ExitStack() as ctx:
        # ---------------- persistent pools ----------------
        pp = ctx.enter_context(tc.tile_pool(name="persist", bufs=1))
        gates = ctx.enter_context(tc.tile_pool(name="gates", bufs=3))
        htp = ctx.enter_context(tc.tile_pool(name="htp", bufs=2))
        xbp = ctx.enter_context(tc.tile_pool(name="xbp", bufs=2))
        ident = pp.tile([128, 128], BF16)
        nc.sync.dma_start(ident[:], dr["ident"][:, :])
        ones_bf = pp.tile([1, 128], BF16)
        nc.vector.memset(ones_bf[:], 1.0)

        # input-derived tiles
        main1 = pp.tile([128, BS], F32)   # input_main[t, b], t 0..127
        main2 = pp.tile([40, BS], F32)    # t 128..167
        nc.sync.dma_start(main1[:], x_in[:, 0:128, 0].rearrange("b t -> t b"))
        nc.sync.dma_start(main2[:], x_in[:, 128:L, 0].rearrange("b t -> t b"))

        fcb_f_sb = pp.tile([1, 1], BF16)
        nc.vector.memset(fcb_f_sb[:], fcb_f)
        fcb_e_sb = pp.tile([1, 1], BF16)
        nc.vector.memset(fcb_e_sb[:], fcb_e)

        h_f = pp.tile([128, H], BF16)
        h_e = pp.tile([128, H], BF16)

        # ---------------- TCN encoder (last timestep only) ----------------
        with tc.tile_pool(name="wconv", bufs=1) as wc, \
             tc.tile_pool(name="cpsum", bufs=1, space="PSUM") as cp, \
             tc.tile_pool(name="c1psum", bufs=2, space="PSUM") as cp1:
            w1t_sb = wc.tile([24, H], BF16)
            nc.sync.dma_start(w1t_sb[:], dr["w1t"][:, :])
            w2r_sb = wc.tile([128, KW, NB, H], BF16)
            nc.sync.dma_start(w2r_sb[:], dr["w2r"][:, :, :, :].rearrange("t k p n -> p t k n"))
            downt_sb = wc.tile([F, H], BF16)
            nc.sync.dma_start(downt_sb[:], dr["downt"][:, :])
            b1_sb = wc.tile([128, NB], F32)
            nc.sync.dma_start(b1_sb[:], dr["b1"][:, :].rearrange("m p -> p m"))
            b2row_sb = wc.tile([1, H], BF16)
            nc.sync.dma_start(b2row_sb[:], dr["b2row"][:, :])
            bdrow_sb = wc.tile([1, H], BF16)
            nc.sync.dma_start(bdrow_sb[:], dr["bdrow"][:, :])

            # window tiles: xw[tp] = x^T rows (k*8+f) for out-time 165+tp
            xw = []
            for tp in range(3):
                t0 = L - 5 + tp  # 163+tp
                xf = wc.tile([24, BS], F32, tag=f"xwf{tp}", name=f"xwf{tp}")
                nc.sync.dma_start(xf[:], x_in[:, t0:t0 + 3, :].rearrange("b t f -> (t f) b"))
                xb_ = wc.tile([24, BS], BF16, tag=f"xwb{tp}", name=f"xwb{tp}")
                nc.scalar.copy(xb_[:], xf[:])
                xw.append(xb_)
            # x(L-1)^T for the 1x1 downsample (needs base_partition 0)
            xlastf = wc.tile([F, BS], F32)
            nc.sync.dma_start(xlastf[:], x_in[:, L - 1, :].rearrange("b f -> f b"))
            xlast = wc.tile([F, BS], BF16)
            nc.scalar.copy(xlast[:], xlastf[:])

            # conv1: out1T[tp] in bf16, [128, 3, 8, 128] (p, tap, mb, b)
            o1 = wc.tile([128, KW, NB, BS], BF16)
            for tp in range(3):
                for mb in range(NB):
                    ps1 = cp1.tile([128, BS], F32)
                    nc.tensor.matmul(ps1[:], w1t_sb[:, mb * 128:(mb + 1) * 128], xw[tp][:], start=True, stop=True)
                    nc.scalar.activation(o1[:, tp, mb, :], ps1[:], AF.Relu, bias=b1_sb[:, mb:mb + 1])

            # conv2 (+bias via ones-row) and downsample residual
            c2 = cp.tile([128, H], F32)
            rp = cp.tile([128, H], F32)
            for nch in range(2):
                cols = slice(nch * 512, (nch + 1) * 512)
                nc.tensor.matmul(c2[:, cols], ones_bf[:], b2row_sb[:, cols], start=True, stop=False)
                for tp in range(3):
                    for kb in range(NB):
                        nc.tensor.matmul(c2[:, cols], o1[:, tp, kb, :], w2r_sb[:, tp, kb, cols],
                                         start=False, stop=(tp == 2 and kb == NB - 1))
                nc.tensor.matmul(rp[:, cols], ones_bf[:], bdrow_sb[:, cols], start=True, stop=False)
                nc.tensor.matmul(rp[:, cols], xlast[:], downt_sb[:, cols], start=False, stop=True)

            o2 = wc.tile([128, H], F32)
            nc.scalar.activation(o2[:], c2[:], AF.Relu)
            hidpre = wc.tile([128, H], F32)
            nc.vector.tensor_add(hidpre[:], o2[:], rp[:])
            nc.scalar.activation(h_f[:], hidpre[:], AF.Relu)
            nc.scalar.activation(h_e[:], hidpre[:], AF.Relu)

        # ---------------- GRU-phase PSUM pools ----------------
        gpsum = ctx.enter_context(tc.tile_pool(name="gpsum", bufs=3, space="PSUM"))
        tpsum = ctx.enter_context(tc.tile_pool(name="tpsum", bufs=3, space="PSUM"))
        ppsum = ctx.enter_context(tc.tile_pool(name="ppsum", bufs=2, space="PSUM"))

        # initial hT (same for both chains)
        hT_f = htp.tile([128, NB, BS], BF16, tag="hT_f")
        hT_e = htp.tile([128, NB, BS], BF16, tag="hT_e")
        for blk in range(NB):
            tp_ = tpsum.tile([128, 128], BF16)
            nc.tensor.transpose(tp_[:], h_f[:, blk * 128:(blk + 1) * 128], ident[:])
            nc.scalar.copy(hT_f[:, blk, :], tp_[:])
            nc.scalar.copy(hT_e[:, blk, :], tp_[:])

        # x0 rows: f_x0 = inputs[:,167,0]; e_x0 = inputs[:,0,0]
        fx0 = pp.tile([1, BS], F32)
        nc.sync.dma_start(fx0[:], x_in[:, L - 1:L, 0].rearrange("b t -> t b"))
        ex0 = pp.tile([1, BS], F32)
        nc.sync.dma_start(ex0[:], x_in[:, 0:1, 0].rearrange("b t -> t b"))
        xb_f = xbp.tile([2, BS], BF16, tag="xb_f")
        nc.vector.memset(xb_f[0:2, :], 1.0)
        nc.scalar.copy(xb_f[0:1, :], fx0[:])
        xb_e = xbp.tile([2, BS], BF16, tag="xb_e")
        nc.vector.memset(xb_e[0:2, :], 1.0)
        nc.scalar.copy(xb_e[0:1, :], ex0[:])

        # ---------------- GRU weights ----------------
        wg = ctx.enter_context(tc.tile_pool(name="wgru", bufs=1))
        wh = {}
        wxb = {}
        fc = {}
        for s in ("f", "e"):
            wh[s] = wg.tile([128, NB, 3 * H], BF16, tag=f"wh_{s}", name=f"wh_{s}")
            nc.sync.dma_start(wh[s][:], dr[f"wh_{s}"][:, :, :].rearrange("k p c -> p k c"))
            wxb[s] = wg.tile([2, 4 * H], BF16, tag=f"wxb_{s}", name=f"wxb_{s}")
            nc.sync.dma_start(wxb[s][:], dr[f"wxb_{s}"][:, :])
            fc[s] = wg.tile([128, NB], BF16, tag=f"fc_{s}", name=f"fc_{s}")
            nc.sync.dma_start(fc[s][:], dr[f"fc_{s}"][:, :].rearrange("k p -> p k"))

        state = {
            "f": {"h": h_f, "hT": hT_f, "xb": xb_f, "fcb": fcb_f_sb, "raw": fore_o},
            "e": {"h": h_e, "hT": hT_e, "xb": xb_e, "fcb": fcb_e_sb, "raw": predse_raw},
        }

        def gru_step(s, t):
            S = state[s]
            newhT = htp.tile([128, NB, BS], BF16, tag=f"hT_{s}")
            for blk in range(NB):
                ps = gpsum.tile([128, 512], F32, tag="gps")
                c0 = blk * 512
                nc.tensor.matmul(ps[:, 0:384], S["xb"][:], wxb[s][:, c0:c0 + 384], start=True, stop=False)
                for k in range(NB):
                    nc.tensor.matmul(ps[:, 0:384], S["hT"][:, k, :], wh[s][:, k, blk * 384:(blk + 1) * 384],
                                     start=False, stop=(k == NB - 1))
                nc.tensor.matmul(ps[:, 384:512], S["xb"][:], wxb[s][:, c0 + 384:c0 + 512], start=True, stop=True)

                cols = slice(blk * 128, (blk + 1) * 128)
                r = gates.tile([128, 128], F32, tag="r")
                nc.scalar.activation(r[:], ps[:, 0:128], AF.Sigmoid)
                z = gates.tile([128, 128], F32, tag="z")
                nc.scalar.activation(z[:], ps[:, 128:256], AF.Sigmoid)
                v = gates.tile([128, 128], F32, tag="v")
                nc.vector.tensor_mul(v[:], r[:], ps[:, 256:384])
                w_ = gates.tile([128, 128], F32, tag="w")
                nc.vector.tensor_add(w_[:], v[:], ps[:, 384:512])
                n = gates.tile([128, 128], F32, tag="n")
                nc.scalar.activation(n[:], w_[:], AF.Tanh)
                d = gates.tile([128, 128], F32, tag="d")
                nc.vector.tensor_sub(d[:], S["h"][:, cols], n[:])
                e_ = gates.tile([128, 128], F32, tag="e")
                nc.vector.tensor_mul(e_[:], d[:], z[:])
                nc.vector.tensor_add(S["h"][:, cols], e_[:], n[:])

                tp_ = tpsum.tile([128, 128], BF16)
                nc.tensor.transpose(tp_[:], S["h"][:, cols], ident[:])
                nc.scalar.copy(newhT[:, blk, :], tp_[:])

            pps = ppsum.tile([1, BS], F32)
            nc.tensor.matmul(pps[:], S["fcb"][:], ones_bf[:], start=True, stop=False)
            for k in range(NB):
                nc.tensor.matmul(pps[:], fc[s][:, k:k + 1], newhT[:, k, :], start=False, stop=(k == NB - 1))
            prow = gates.tile([1, BS], F32, tag="prow")
            nc.scalar.copy(prow[:], pps[:])
            nc.sync.dma_start(S["raw"][t:t + 1, :], prow[:])
            newxb = xbp.tile([2, BS], BF16, tag=f"xb_{s}")
            nc.vector.memset(newxb[0:2, :], 1.0)
            nc.scalar.copy(newxb[0:1, :], pps[:])
            S["hT"] = newhT
            S["xb"] = newxb

        if INTERLEAVE:
            for t in range(max(T_EST, T_FORE)):
                if t < T_EST:
                    gru_step("e", t)
                if t < T_FORE:
                    gru_step("f", t)
        else:
            for t in range(T_FORE):
                gru_step("f", t)
            for t in range(T_EST):
                gru_step("e", t)

        # ---------------- outputs (in [T, B] layout; host transposes) ----------------
        ne1 = min(T_EST, 128)
        ep1 = pp.tile([128, BS], F32)
        nc.sync.dma_start(ep1[0:ne1, :], predse_raw[0:ne1, :])
        est1 = pp.tile([128, BS], F32)
        nc.vector.tensor_sub(est1[0:ne1, :], main1[0:ne1, :], ep1[0:ne1, :])
        nc.sync.dma_start(est_o[0:ne1, :], est1[0:ne1, :])
        if T_EST > 128:
            ep2 = pp.tile([40, BS], F32)
            nc.sync.dma_start(ep2[0:T_EST - 128, :], predse_raw[128:T_EST, :])
            est2 = pp.tile([40, BS], F32)
            nc.vector.tensor_sub(est2[0:T_EST - 128, :], main2[0:T_EST - 128, :], ep2[0:T_EST - 128, :])
            nc.sync.dma_start(est_o[128:T_EST, :], est2[0:T_EST - 128, :])

    _split_excess_waits(nc, maxw=1)
    nc.finalize()
    return nc


def _split_excess_waits(nc, maxw=1):
    """Walrus codegen only supports a limited number of sync-wait commands
    per instruction; hoist the excess onto NoOps on the same engine."""
    nid = 0
    for fn in nc.m.functions:
        for bb in fn.blocks:
            newl = []
            changed = False
            for ins in bb.instructions:
                si = ins.sync_info
                if si is not None and len(si.on_wait) > maxw:
                    waits = list(si.on_wait)
                    excess, keep = waits[:-maxw], waits[-maxw:]
                    for i in range(0, len(excess), maxw):
                        n = mybir.InstNoOp(name=f"I-wsplit-{nid}", ins=[], outs=[])
                        nid += 1
                        n.engine = ins.engine
                        n.sync_info = bass_rust.SyncInfo(on_wait=excess[i:i + maxw], on_update=[])
                        newl.append(n)
                    ins.sync_info = bass_rust.SyncInfo(on_wait=keep, on_update=list(si.on_update))
                    changed = True
                newl.append(ins)
            if changed:
                bb.instructions = newl


def kernel(**inputs):
    p = {k: np.asarray(v, dtype=np.float32) for k, v in inputs.items()}
    w = _prep_weights(p)
    nc = _build(float(p["f_fc_b"][0]), float(p["e_fc_b"][0]))

    x = p["inputs"]  # [1024, 168, 8]
    in_maps = []
    for c in range(NCORES):
        m = {"x_in": np.ascontiguousarray(x[c * BS:(c + 1) * BS])}
        for k, v in w.items():
            m[k] = v
        in_maps.append(m)

    res = run_bass_kernel_spmd(nc, in_maps, list(range(NCORES))).results
    fore = np.concatenate([res[c]["fore"].T for c in range(NCORES)], axis=0)[:, :, None]
    esto = np.concatenate([res[c]["esto"].T for c in range(NCORES)], axis=0)[:, :, None]
    return fore.astype(np.float32), esto.astype(np.float32)


if __name__ == "__main__":
    rng = np.random.default_rng(0)
    fake = {"inputs": rng.standard_normal((B, L, F), dtype=np.float32)}
    # quick structural build test only
    print("building...")
    nc = _build(0.0, 0.0)
    print("built OK")
